# revision 21
# baseline (speedup 1.0000x reference)
"""Causal multi-head attention (RoPE) forward for Trainium2, 8 NeuronCores.

Problem: B=2, T=2048, C=1024, H=16, D=64.  out = proj(softmax(rope(q) rope(k)^T / 8, causal) @ v)

Sharding: 8 cores = 2 batches x 4 head-groups (4 heads each).
 - qkv projection column-sharded per head group, proj row-sharded; host sums
   the 4 per-group partial projections per batch (partials shipped as bf16).
 - Matmul cost on PE is proportional to OUTPUT free-size only, so the AV
   matmul runs in transposed layout (queries on partitions, 65 output
   columns per head): per 128-query tile and key tile,
     y_aug[i, 65h:65h+65] += pts[:, it]^T @ v_aug[:, jb, h]   (PSUM accum)
   with the ones column of v_aug producing the softmax denominator as a
   per-partition scalar -- normalization is then a single DVE multiply with
   a broadcast reciprocal, no rank-1 broadcast matmul needed.
 - Layouts:
     q^T,k^T   [d, t]  <- w^T-block as lhsT, x^T as rhs
     v         [t, d]  <- x^T-block as lhsT, w^T v-cols as rhs
     S^T       [j, i]  <- k^T-tile as lhsT, q^T as rhs       (j keys, i queries)
     P^T       = exp(S^T/8)  (no max subtraction: |scores| <= ~3 by construction)
     y_aug     [i, 65] <- P^T i-tile as lhsT, v_aug as rhs, PSUM-accumulated over j
     y^T       [c, i]  <- PE transpose of normalized y (bf16)
     out       [t, o]  <- y^T-tile as lhsT, wproj^T as rhs
 - RoPE: q_rope = q*cos + R(q*sinP), with sinP a half-swapped sin table and
   R the block-diagonal rotate-half matrix applied by one 128x128 PE matmul
   per tile.
 - Causality: key-tiles above the diagonal are skipped; on diagonal tiles
   QK/exp compute only the surviving query range and a single triangular
   [128,128] mask is applied after the exp; AV skips dead (i-tile, j-tile)
   pairs entirely.
 - Single fused chunk pipeline: qkv(tc) -> rope(tc) -> attention(ic=tc) with
   the output projection deferred to fill tail PE gaps.
"""

import numpy as np
import ml_dtypes

_CACHE = {}

B, T, C = 2, 2048, 1024
HLOC, D = 4, 64            # heads per core, head dim
GC = HLOC * D              # 256 channels per group
P = 128
NTT = T // P               # 16 key/row tiles
NIC = T // 512             # 4 query chunks of 512
THETA = 10000.0
N_CORES = 8


def _rope_tables():
    freqs = 1.0 / THETA ** (np.arange(0, D, 2, dtype=np.float32) / D)
    t = np.arange(T, dtype=np.float32)
    f = np.outer(t, freqs)                          # [T, 32]
    emb = np.concatenate([f, f], axis=-1)           # [T, 64]
    cosT = np.cos(emb).T.astype(np.float32)         # [64, T]
    sinT = np.sin(emb).T.astype(np.float32)
    # tile to 128 partitions (2 heads per partition block)
    return (np.concatenate([cosT, cosT], 0), np.concatenate([sinT, sinT], 0))


def _build_program():
    import concourse.bass as bass
    import concourse.mybir as mybir
    import concourse.tile as tile

    dt = mybir.dt
    fp32 = dt.float32
    bf16 = dt.bfloat16
    EXP = mybir.ActivationFunctionType.Exp
    MUL = mybir.AluOpType.mult

    nc = bass.Bass("TRN2", target_bir_lowering=False, debug=False,
                   enable_asserts=True, num_devices=N_CORES)

    xT = nc.dram_tensor("xT", [C, T], bf16, kind="ExternalInput").ap()
    wT = nc.dram_tensor("wT", [C, 3 * GC], bf16, kind="ExternalInput").ap()        # q,k,v col-blocks
    rmat_d = nc.dram_tensor("rmat", [P, P], bf16, kind="ExternalInput").ap()
    ident_d = nc.dram_tensor("ident", [P, P], bf16, kind="ExternalInput").ap()
    wpT = nc.dram_tensor("wpT", [GC, C], bf16, kind="ExternalInput").ap()
    cosT_d = nc.dram_tensor("cosT", [P, T], bf16, kind="ExternalInput").ap()
    sinT_d = nc.dram_tensor("sinT", [P, T], bf16, kind="ExternalInput").ap()
    tri_d = nc.dram_tensor("tri", [P, P], bf16, kind="ExternalInput").ap()
    out_d = nc.dram_tensor("out", [T, C], bf16, kind="ExternalOutput").ap()

    CO = C // P  # 8 contraction blocks

    with tile.TileContext(nc) as tc:
        with (
            tc.tile_pool(name="persist", bufs=1) as persist,
            tc.tile_pool(name="work", bufs=8) as work,
            tc.tile_pool(name="pt", bufs=6) as ptpool,
            tc.tile_pool(name="outp", bufs=8) as outpool,
            tc.tile_pool(name="wk", bufs=2, space="PSUM") as wkp,
            tc.tile_pool(name="sp", bufs=2, space="PSUM") as spp,
            tc.tile_pool(name="ya", bufs=2, space="PSUM") as yap,
        ):
            # ---- persistent SBUF loads -------------------------------------
            # Few big strided DMAs spread across HWDGE queues (SP + ACT + DVE)
            # so tiles land in first-use order without per-tile queue overhead.
            warm_w = persist.tile([P, P], bf16, tag="warm_w")
            nc.gpsimd.memset(warm_w[:], 0.0)
            warm = wkp.tile([P, P], fp32, tag="wk", name="warmup")
            for i in range(56):
                nc.tensor.matmul(warm[:], warm_w[:], warm_w[:],
                                 start=True, stop=True, skip_group_check=True)
            # All input DMAs on the SP queue in need-order: transfers serialize
            # on the DMA engines, so queue order == arrival order.
            rmat_sb = persist.tile([P, P], bf16, tag="rmat")
            nc.sync.dma_start(rmat_sb[:], rmat_d[:])
            x_sb = [persist.tile([P, CO, 512], bf16, tag=f"x{i}", name=f"x{i}")
                    for i in range(NIC)]
            def load_x(tcix, half):
                co0, co1 = (0, 4) if half == 0 else (4, CO)
                src = xT[P * co0:P * co1, 512 * tcix:512 * (tcix + 1)].rearrange(
                    "(co p) t -> p co t", p=P)
                nc.sync.dma_start(x_sb[tcix][:, co0:co1], src)
            wqk_sb = persist.tile([P, CO, 512], bf16, tag="wqk")
            nc.sync.dma_start(wqk_sb[:, 0:4],
                              wT[0:512, 0:512].rearrange("(co p) q -> p co q", p=P))
            load_x(0, 0)
            nc.sync.dma_start(wqk_sb[:, 4:8],
                              wT[512:1024, 0:512].rearrange("(co p) q -> p co q", p=P))
            load_x(0, 1)
            cos_sb = persist.tile([P, T], bf16, tag="cos")
            nc.sync.dma_start(cos_sb[:], cosT_d[:])
            sin_sb = persist.tile([P, T], bf16, tag="sin")
            nc.sync.dma_start(sin_sb[:], sinT_d[:])
            wrv_sb = persist.tile([P, CO, 256], bf16, tag="wrv")
            nc.sync.dma_start(wrv_sb[:], wT[:, 512:768].rearrange("(co p) q -> p co q", p=P))
            load_x(1, 0)
            load_x(1, 1)
            tri_sb = persist.tile([P, P], bf16, tag="tri")
            nc.sync.dma_start(tri_sb[:], tri_d[:])
            ident_sb = persist.tile([P, P], bf16, tag="ident")
            nc.sync.dma_start(ident_sb[:], ident_d[:])
            wpT_sb = persist.tile([P, 2, C], bf16, tag="wpT")
            nc.sync.dma_start(wpT_sb[:], wpT.rearrange("(cb p) o -> p cb o", p=P))
            load_x(2, 0)
            load_x(2, 1)
            load_x(3, 0)
            load_x(3, 1)

            # rope outputs: q^T,k^T per 2-head block [128, T] bf16
            qk_rope = [persist.tile([P, T], bf16, tag=f"qkrope{i}", name=f"qkrope{i}") for i in range(4)]
            # v with ones column per head: [128part=t, 16 ttiles, 4*65]
            v_aug = persist.tile([P, NTT, HLOC * (D + 1)], bf16, tag="vaug")
            nc.vector.memset(v_aug[:], 1.0)
            # normalized y^T blocks (2 channel blocks of 128) [128, T] bf16
            ynT = [persist.tile([P, T], bf16, tag=f"ynT{i}", name=f"ynT{i}") for i in range(2)]

            def emit_proj(tt):
                for oc in range(2):
                    ps = wkp.tile([P, 512], fp32, tag="wk", name=f"pso_{tt}_{oc}")
                    for cb in range(2):
                        nc.tensor.matmul(
                            ps[:], ynT[cb][:, 128 * tt:128 * (tt + 1)],
                            wpT_sb[:, cb, 512 * oc:512 * (oc + 1)],
                            start=(cb == 0), stop=(cb == 1))
                    ob = outpool.tile([P, 512], bf16, tag="ob")
                    nc.gpsimd.tensor_copy(out=ob[:], in_=ps[:])
                    nc.sync.dma_start(out_d[128 * tt:128 * (tt + 1), 512 * oc:512 * (oc + 1)], ob[:])

            pend_tp = []

            def flush_tp():
                # y^T transposes ride the wk PSUM slots, flushed a chunk late:
                # their norm deps are long resolved, so they rotate through
                # without stalling the next chunk's qkv
                for tt, yn in pend_tp:
                    for cb in range(2):
                        tp = wkp.tile([P, P], bf16, tag="wk", name=f"tp_{tt}_{cb}")
                        nc.tensor.transpose(tp[:], yn[:, 128 * cb:128 * (cb + 1)],
                                            ident_sb[:])
                        nc.vector.tensor_copy(
                            out=ynT[cb][:, 128 * tt:128 * (tt + 1)], in_=tp[:])
                pend_tp.clear()

            # ---- fused pipeline: per 512-token chunk tc:
            #   qkv(tc) -> rope(tc) -> attention(ic=tc, keys<=tc)
            # (causality means chunk tc's attention needs only k/v chunks <= tc,
            #  so the ACT-bound softmax overlaps the PE-bound qkv of later chunks)
            for tcix in range(NIC):
                flush_tp()
                # q/k projections (ft 0,1 q; 2,3 k) + rotated partners, + rope
                for ft in range(4):
                    ps = wkp.tile([P, 512], fp32, tag="wk", name=f"psq_{ft}_{tcix}")
                    for co in range(CO):
                        nc.tensor.matmul(
                            ps[:], wqk_sb[:, co, 128 * ft:128 * (ft + 1)],
                            x_sb[tcix][:, co], start=(co == 0), stop=(co == CO - 1))
                    t1 = work.tile([P, 512], bf16, tag="t1")
                    nc.vector.tensor_tensor(t1[:], ps[:], cos_sb[:, 512 * tcix:512 * (tcix + 1)], MUL)
                    u = work.tile([P, 512], bf16, tag="u")
                    nc.vector.tensor_tensor(u[:], ps[:], sin_sb[:, 512 * tcix:512 * (tcix + 1)], MUL)
                    psr = wkp.tile([P, 512], fp32, tag="wk", name=f"psr_{ft}_{tcix}")
                    nc.tensor.matmul(psr[:], rmat_sb[:], u[:], start=True, stop=True)
                    nc.vector.tensor_add(qk_rope[ft][:, 512 * tcix:512 * (tcix + 1)], psr[:], t1[:])
                # v for this chunk's 4 key tiles
                for tt in range(4 * tcix, 4 * tcix + 4):
                    ps = wkp.tile([P, 512], fp32, tag="wk", name=f"psv_{tt}")
                    for co in range(CO):
                        nc.tensor.matmul(
                            ps[:, :GC], x_sb[tt // 4][:, co, 128 * (tt % 4):128 * (tt % 4 + 1)],
                            wrv_sb[:, co], start=(co == 0), stop=(co == CO - 1))
                    nc.gpsimd.tensor_copy(
                        out=v_aug[:, tt].rearrange("p (h e) -> p h e", e=D + 1)[:, :, :D],
                        in_=ps[:, :GC].rearrange("p (h d) -> p h d", d=D))

                # attention for query chunk ic=tcix, in query halves of 256
                # (4 key tiles of 256 i-cols share one 1024-wide span/exp)
                ic = tcix
                for ih in range(2):
                    njb = 4 * ic + 2 * ih + 2        # causal: last key tile 4ic+2ih+1
                    yac = [yap.tile([P, HLOC * (D + 1)], fp32, tag="ya",
                                    name=f"yac_{ic}_{ih}_{itl}") for itl in range(2)]
                    for hp in range(2):              # head pair = partition block
                        for u2 in range(2):
                            h = 2 * hp + u2
                            hb = 64 * u2
                            for s in range((njb + 3) // 4):
                                jbs = list(range(4 * s, min(4 * s + 4, njb)))
                                sp = spp.tile([P, 1024], fp32, tag="sp",
                                              name=f"sp_{ic}_{ih}_{h}_{s}")
                                pt = ptpool.tile([P, 1024], bf16, tag="pt",
                                                 name=f"pt_{ic}_{ih}_{h}_{s}")
                                qos = []
                                for c, jb in enumerate(jbs):
                                    m = jb - 4 * ic
                                    qo = max(0, 128 * m - 256 * ih)
                                    qos.append(qo)
                                    nc.tensor.matmul(
                                        sp[:, 256 * c + qo:256 * (c + 1)],
                                        qk_rope[2 + hp][hb:hb + 64, 128 * jb:128 * (jb + 1)],
                                        qk_rope[hp][hb:hb + 64,
                                                    512 * ic + 256 * ih + qo:
                                                    512 * ic + 256 * (ih + 1)],
                                        start=True, stop=True)
                                # exp over contiguous runs (qo>0 breaks a run)
                                runs, r0 = [], 0
                                for c, qo in enumerate(qos):
                                    if qo > 0:
                                        if 256 * c > r0:
                                            runs.append((r0, 256 * c))
                                        r0 = 256 * c + qo
                                if 256 * len(qos) > r0:
                                    runs.append((r0, 256 * len(qos)))
                                for (a, b) in runs:
                                    nc.scalar.activation(pt[:, a:b], sp[:, a:b],
                                                         EXP, scale=0.125)
                                for c, jb in enumerate(jbs):
                                    m = jb - 4 * ic
                                    for itl in range(2):
                                        it = 2 * ih + itl
                                        if it < m:
                                            continue
                                        if it == m:  # diagonal tile -> causal mask
                                            nc.vector.tensor_tensor(
                                                pt[:, 256 * c + 128 * itl:
                                                   256 * c + 128 * (itl + 1)],
                                                pt[:, 256 * c + 128 * itl:
                                                   256 * c + 128 * (itl + 1)],
                                                tri_sb[:], MUL)
                                        nc.tensor.matmul(
                                            yac[itl][:, 65 * h:65 * (h + 1)],
                                            pt[:, 256 * c + 128 * itl:
                                               256 * c + 128 * (itl + 1)],
                                            v_aug[:, jb, 65 * h:65 * (h + 1)],
                                            start=(jb == 0),
                                            stop=(jb == 4 * ic + it))
                    # normalize per 128-query tile; defer the transposes so
                    # their sp-slot turns don't stall the next half's spans
                    for itl in range(2):
                        it = 2 * ih + itl
                        tt = 4 * ic + it
                        yv = yac[itl].rearrange("p (h e) -> p h e", e=D + 1)
                        rec = work.tile([P, HLOC], fp32, tag="rec", name=f"rec_{tt}")
                        nc.vector.reciprocal(rec[:], yv[:, :, D])
                        yn = work.tile([P, GC], bf16, tag="yn", name=f"yn_{tt}",
                                       bufs=10)
                        nc.vector.tensor_tensor(
                            yn[:].rearrange("p (h d) -> p h d", d=D),
                            yv[:, :, :D], rec[:].broadcast_to((P, HLOC, D)), MUL)
                        pend_tp.append((tt, yn))
            flush_tp()
            # output projection emitted last (= lowest scheduler priority):
            # its matmuls fill PE gaps in the ACT-paced attention stretches
            for tt in range(NTT):
                emit_proj(tt)


    _split_excess_waits(nc)
    return nc


def _split_excess_waits(nc, maxw=1):
    """Walrus codegen rejects instructions carrying >1 sem wait; move excess
    waits onto no-ops inserted immediately before, on the same engine."""
    import concourse.mybir as mybir
    n = 0
    for f in nc.m.functions:
        for bb in f.blocks:
            new = []
            for inst in bb.instructions:
                si = getattr(inst, "sync_info", None)
                if si is not None and si.on_wait and len(si.on_wait) > maxw:
                    waits = list(si.on_wait)
                    excess, keep = waits[:-maxw], waits[-maxw:]
                    for i in range(0, len(excess), maxw):
                        new.append(mybir.InstNoOp(
                            name=f"{inst.name}_wsp{n}_{i}", engine=inst.engine,
                            bass_nofuse=True,
                            sync_info=mybir.SyncInfo(on_wait=excess[i:i + maxw],
                                                     on_update=[])))
                    si.on_wait = keep
                    n += 1
                new.append(inst)
            bb.instructions[:] = new
    return n


def _get_runner():
    """Build the Bass program once and wrap it in a shard_map-jitted callable
    over the 8 cores (mirrors concourse.bass2jax.run_bass_via_pjrt)."""
    if "runner" in _CACHE:
        return _CACHE["runner"]
    import jax
    import numpy as _np
    from jax.sharding import Mesh, PartitionSpec
    from jax.experimental.shard_map import shard_map
    import concourse.mybir as mybir
    from concourse.bass2jax import _bass_exec_p, install_neuronx_cc_hook

    install_neuronx_cc_hook()
    from concourse.bass2jax import partition_id_tensor
    nc = _build_program()

    part_name = nc.partition_id_tensor.name if nc.partition_id_tensor else None
    in_names, out_names, out_avals = [], [], []
    for alloc in nc.m.functions[0].allocations:
        if not isinstance(alloc, mybir.MemoryLocationSet):
            continue
        name = alloc.memorylocations[0].name
        if alloc.kind == "ExternalInput":
            if name != part_name:
                in_names.append(name)
        elif alloc.kind == "ExternalOutput":
            out_names.append(name)
            out_avals.append(jax.core.ShapedArray(
                tuple(alloc.tensor_shape), mybir.dt.np(alloc.dtype)))
    n_params = len(in_names)
    all_names = in_names + out_names
    if part_name is not None:
        all_names = all_names + [part_name]

    def _body(*args):
        operands = list(args)
        if part_name is not None:
            operands.append(partition_id_tensor())
        outs = _bass_exec_p.bind(
            *operands, out_avals=tuple(out_avals), in_names=tuple(all_names),
            out_names=tuple(out_names), lowering_input_output_aliases=(),
            sim_require_finite=True, sim_require_nnan=True, nc=nc)
        return tuple(outs)

    devices = jax.devices()[:N_CORES]
    mesh = Mesh(_np.asarray(devices), ("core",))
    n_outs = len(out_names)
    sharded = jax.jit(
        shard_map(_body, mesh=mesh,
                  in_specs=(PartitionSpec("core"),) * (n_params + n_outs),
                  out_specs=(PartitionSpec("core"),) * n_outs,
                  check_rep=False),
        donate_argnums=tuple(range(n_params, n_params + n_outs)),
        keep_unused=True)

    runner = (sharded, in_names, out_names, out_avals)
    _CACHE["runner"] = runner
    return runner


def _prepare_core_inputs(x, w_qkv, w_proj):
    bf = ml_dtypes.bfloat16
    cosT, sinT = _CACHE.setdefault("rope", _rope_tables())
    # q_rope = q*cos + R(q * sinP) with sinP = half-swapped sin:
    #   (R(q*sinP))[d] = sign_d * q[s(d)] * sinP[s(d)] = rot_half(q)[d] * sin[d]
    sinP = np.concatenate([sinT[D // 2:D], sinT[:D // 2]], axis=0)
    sinP = np.concatenate([sinP, sinP], axis=0)[:P]
    cosT, sinT = cosT.astype(bf), sinP.astype(bf)
    # lhsT for the on-device rotate-half matmul: out = rmat.T @ q = R_pair @ q
    R = np.zeros((D, D), np.float32)
    for d in range(D // 2):
        R[d, d + D // 2] = -1.0
        R[d + D // 2, d] = 1.0
    R_pair = np.zeros((P, P), np.float32)
    R_pair[:D, :D] = R
    R_pair[D:, D:] = R
    rmat = np.ascontiguousarray(R_pair.T).astype(bf)
    ident = np.eye(P, dtype=np.float32).astype(bf)
    # tri[j, i'] = 1 if i' >= j  (keep keys at or before the query)
    tri = np.triu(np.ones((P, P), np.float32)).astype(bf)
    xTs = [np.ascontiguousarray(x[b].T).astype(bf) for b in range(B)]
    per_core = []
    for core in range(N_CORES):
        b, g = divmod(core, 4)
        rows = slice(GC * g, GC * (g + 1))
        wq = w_qkv[0 * C:1 * C][rows]
        wk = w_qkv[1 * C:2 * C][rows]
        wv = w_qkv[2 * C:3 * C][rows]
        wT = np.ascontiguousarray(
            np.concatenate([wq, wk, wv], axis=0).T).astype(bf)      # [C, 768]
        wpT = np.ascontiguousarray(w_proj[:, rows].T).astype(bf)    # [256, C]
        per_core.append({
            "xT": xTs[b], "wT": wT, "wpT": wpT, "rmat": rmat,
            "ident": ident, "cosT": cosT, "sinT": sinT, "tri": tri})
    return per_core


def _run_cores(per_core):
    from concourse import bass_utils
    if "nc" not in _CACHE:
        from concourse.bass2jax import install_neuronx_cc_hook
        install_neuronx_cc_hook()
        _CACHE["nc"] = _build_program()
    res = bass_utils.run_bass_kernel_spmd(
        _CACHE["nc"], per_core, core_ids=list(range(N_CORES)))
    return res.results


def kernel(x, w_qkv, w_proj):
    x = np.asarray(x, dtype=np.float32)
    w_qkv = np.asarray(w_qkv, dtype=np.float32)
    w_proj = np.asarray(w_proj, dtype=np.float32)
    per_core = _prepare_core_inputs(x, w_qkv, w_proj)
    results = _run_cores(per_core)
    out = np.zeros((B, T, C), dtype=np.float32)
    for core in range(N_CORES):
        b = core // 4
        out[b] += np.asarray(results[core]["out"], dtype=np.float32)
    return out


# revision 27
# speedup vs baseline: 1.2808x; 1.2808x over previous
"""Causal multi-head attention (RoPE) forward for Trainium2, 8 NeuronCores.

Problem: B=2, T=2048, C=1024, H=16, D=64.  out = proj(softmax(rope(q) rope(k)^T / 8, causal) @ v)

Sharding: 8 cores = 2 batches x 4 head-groups (4 heads each).
 - qkv projection column-sharded per head group, proj row-sharded; host sums
   the 4 per-group partial projections per batch (partials shipped as bf16).
 - Matmul cost on PE is proportional to OUTPUT free-size only, so the AV
   matmul runs in transposed layout (queries on partitions, 65 output
   columns per head): per 128-query tile and key tile,
     y_aug[i, 65h:65h+65] += pts[:, it]^T @ v_aug[:, jb, h]   (PSUM accum)
   with the ones column of v_aug producing the softmax denominator as a
   per-partition scalar -- normalization is then a single DVE multiply with
   a broadcast reciprocal, no rank-1 broadcast matmul needed.
 - Layouts:
     q^T,k^T   [d, t]  <- w^T-block as lhsT, x^T as rhs
     v         [t, d]  <- x^T-block as lhsT, w^T v-cols as rhs
     S^T       [j, i]  <- k^T-tile as lhsT, q^T as rhs       (j keys, i queries)
     P^T       = exp(S^T/8)  (no max subtraction: |scores| <= ~3 by construction)
     y_aug     [i, 65] <- P^T i-tile as lhsT, v_aug as rhs, PSUM-accumulated over j
     y^T       [c, i]  <- PE transpose of normalized y (bf16)
     out       [t, o]  <- y^T-tile as lhsT, wproj^T as rhs
 - RoPE: q_rope = q*cos + R(q*sinP), with sinP a half-swapped sin table and
   R the block-diagonal rotate-half matrix applied by one 128x128 PE matmul
   per tile.
 - Causality: key-tiles above the diagonal are skipped; on diagonal tiles
   QK/exp compute only the surviving query range and a single triangular
   [128,128] mask is applied after the exp; AV skips dead (i-tile, j-tile)
   pairs entirely.
 - Single fused chunk pipeline: qkv(tc) -> rope(tc) -> attention(ic=tc) with
   the output projection deferred to fill tail PE gaps.
"""

import numpy as np
import ml_dtypes

_CACHE = {}

B, T, C = 2, 2048, 1024
HLOC, D = 4, 64            # heads per core, head dim
GC = HLOC * D              # 256 channels per group
P = 128
NTT = T // P               # 16 key/row tiles
NIC = T // 512             # 4 query chunks of 512
THETA = 10000.0
N_CORES = 8


def _rope_tables():
    freqs = 1.0 / THETA ** (np.arange(0, D, 2, dtype=np.float32) / D)
    t = np.arange(T, dtype=np.float32)
    f = np.outer(t, freqs)                          # [T, 32]
    emb = np.concatenate([f, f], axis=-1)           # [T, 64]
    cosT = np.cos(emb).T.astype(np.float32)         # [64, T]
    sinT = np.sin(emb).T.astype(np.float32)
    # tile to 128 partitions (2 heads per partition block)
    return (np.concatenate([cosT, cosT], 0), np.concatenate([sinT, sinT], 0))


def _build_program():
    import concourse.bass as bass
    import concourse.mybir as mybir
    import concourse.tile as tile

    dt = mybir.dt
    fp32 = dt.float32
    bf16 = dt.bfloat16
    EXP = mybir.ActivationFunctionType.Exp
    MUL = mybir.AluOpType.mult

    nc = bass.Bass("TRN2", target_bir_lowering=False, debug=False,
                   enable_asserts=True, num_devices=N_CORES)

    xT = nc.dram_tensor("xT", [C, T], bf16, kind="ExternalInput").ap()
    wT = nc.dram_tensor("wT", [C, 3 * GC], bf16, kind="ExternalInput").ap()        # q,k,v col-blocks
    rmat_d = nc.dram_tensor("rmat", [P, P], bf16, kind="ExternalInput").ap()
    ident_d = nc.dram_tensor("ident", [P, P], bf16, kind="ExternalInput").ap()
    wpT = nc.dram_tensor("wpT", [GC, C], bf16, kind="ExternalInput").ap()
    cosT_d = nc.dram_tensor("cosT", [P, T], bf16, kind="ExternalInput").ap()
    sinT_d = nc.dram_tensor("sinT", [P, T], bf16, kind="ExternalInput").ap()
    tri_d = nc.dram_tensor("tri", [P, P], bf16, kind="ExternalInput").ap()
    out_d = nc.dram_tensor("out", [T, C], bf16, kind="ExternalOutput").ap()

    CO = C // P  # 8 contraction blocks

    with tile.TileContext(nc) as tc:
        with (
            tc.tile_pool(name="persist", bufs=1) as persist,
            tc.tile_pool(name="work", bufs=8) as work,
            tc.tile_pool(name="pt", bufs=6) as ptpool,
            tc.tile_pool(name="outp", bufs=8) as outpool,
            tc.tile_pool(name="wk", bufs=2, space="PSUM") as wkp,
            tc.tile_pool(name="sp", bufs=2, space="PSUM") as spp,
            tc.tile_pool(name="ya", bufs=2, space="PSUM") as yap,
        ):
            # ---- persistent SBUF loads -------------------------------------
            # Few big strided DMAs spread across HWDGE queues (SP + ACT + DVE)
            # so tiles land in first-use order without per-tile queue overhead.
            warm_w = persist.tile([P, P], bf16, tag="warm_w")
            nc.gpsimd.memset(warm_w[:], 0.0)
            warm = wkp.tile([P, P], fp32, tag="wk", name="warmup")
            for i in range(56):
                nc.tensor.matmul(warm[:], warm_w[:], warm_w[:],
                                 start=True, stop=True, skip_group_check=True)
            # All input DMAs on the SP queue in need-order: transfers serialize
            # on the DMA engines, so queue order == arrival order.
            rmat_sb = persist.tile([P, P], bf16, tag="rmat")
            nc.sync.dma_start(rmat_sb[:], rmat_d[:])
            x_sb = [persist.tile([P, CO, 512], bf16, tag=f"x{i}", name=f"x{i}")
                    for i in range(NIC)]
            def load_x(tcix, half):
                co0, co1 = (0, 4) if half == 0 else (4, CO)
                src = xT[P * co0:P * co1, 512 * tcix:512 * (tcix + 1)].rearrange(
                    "(co p) t -> p co t", p=P)
                nc.sync.dma_start(x_sb[tcix][:, co0:co1], src)
            wqk_sb = persist.tile([P, CO, 512], bf16, tag="wqk")
            nc.sync.dma_start(wqk_sb[:, 0:4],
                              wT[0:512, 0:512].rearrange("(co p) q -> p co q", p=P))
            load_x(0, 0)
            nc.sync.dma_start(wqk_sb[:, 4:8],
                              wT[512:1024, 0:512].rearrange("(co p) q -> p co q", p=P))
            load_x(0, 1)
            load_x(1, 0)
            load_x(1, 1)
            cos_sb = persist.tile([P, T], bf16, tag="cos")
            nc.sync.dma_start(cos_sb[:], cosT_d[:])
            sin_sb = persist.tile([P, T], bf16, tag="sin")
            nc.sync.dma_start(sin_sb[:], sinT_d[:])
            wrv_sb = persist.tile([P, CO, 256], bf16, tag="wrv")
            nc.sync.dma_start(wrv_sb[:], wT[:, 512:768].rearrange("(co p) q -> p co q", p=P))
            tri_sb = persist.tile([P, P], bf16, tag="tri")
            nc.sync.dma_start(tri_sb[:], tri_d[:])
            ident_sb = persist.tile([P, P], bf16, tag="ident")
            nc.sync.dma_start(ident_sb[:], ident_d[:])
            wpT_sb = persist.tile([P, 2, C], bf16, tag="wpT")
            nc.sync.dma_start(wpT_sb[:], wpT.rearrange("(cb p) o -> p cb o", p=P))
            load_x(2, 0)
            load_x(2, 1)
            load_x(3, 0)
            load_x(3, 1)

            # rope outputs: q^T,k^T per 2-head block [128, T] bf16
            qk_rope = [persist.tile([P, T], bf16, tag=f"qkrope{i}", name=f"qkrope{i}") for i in range(4)]
            # v with ones column per head: [128part=t, 16 ttiles, 4*65]
            v_aug = persist.tile([P, NTT, HLOC * (D + 1)], bf16, tag="vaug")
            nc.vector.memset(v_aug[:], 1.0)
            # normalized y^T blocks (2 channel blocks of 128) [128, T] bf16
            ynT = [persist.tile([P, T], bf16, tag=f"ynT{i}", name=f"ynT{i}") for i in range(2)]

            def emit_proj(tt):
                for oc in range(2):
                    ps = wkp.tile([P, 512], fp32, tag="wk", name=f"pso_{tt}_{oc}")
                    for cb in range(2):
                        nc.tensor.matmul(
                            ps[:], ynT[cb][:, 128 * tt:128 * (tt + 1)],
                            wpT_sb[:, cb, 512 * oc:512 * (oc + 1)],
                            start=(cb == 0), stop=(cb == 1))
                    ob = outpool.tile([P, 512], bf16, tag="ob")
                    nc.gpsimd.tensor_copy(out=ob[:], in_=ps[:])
                    nc.sync.dma_start(out_d[128 * tt:128 * (tt + 1), 512 * oc:512 * (oc + 1)], ob[:])

            # ---- fused pipeline: per 512-token chunk tc:
            #   qkv(tc) -> rope(tc) -> attention(ic=tc, keys<=tc)
            # (causality means chunk tc's attention needs only k/v chunks <= tc,
            #  so the ACT-bound softmax overlaps the PE-bound qkv of later chunks)
            for tcix in range(NIC):
                # q/k projections (ft 0,1 q; 2,3 k) + rotated partners, + rope
                for ft in range(4):
                    ps = wkp.tile([P, 512], fp32, tag="wk", name=f"psq_{ft}_{tcix}")
                    for co in range(CO):
                        nc.tensor.matmul(
                            ps[:], wqk_sb[:, co, 128 * ft:128 * (ft + 1)],
                            x_sb[tcix][:, co], start=(co == 0), stop=(co == CO - 1))
                    t1 = work.tile([P, 512], bf16, tag="t1")
                    nc.vector.tensor_tensor(t1[:], ps[:], cos_sb[:, 512 * tcix:512 * (tcix + 1)], MUL)
                    u = work.tile([P, 512], bf16, tag="u")
                    nc.vector.tensor_tensor(u[:], ps[:], sin_sb[:, 512 * tcix:512 * (tcix + 1)], MUL)
                    psr = wkp.tile([P, 512], fp32, tag="wk", name=f"psr_{ft}_{tcix}")
                    nc.tensor.matmul(psr[:], rmat_sb[:], u[:], start=True, stop=True)
                    nc.vector.tensor_add(qk_rope[ft][:, 512 * tcix:512 * (tcix + 1)], psr[:], t1[:])
                # v for this chunk's 4 key tiles
                for tt in range(4 * tcix, 4 * tcix + 4):
                    ps = wkp.tile([P, 512], fp32, tag="wk", name=f"psv_{tt}")
                    for co in range(CO):
                        nc.tensor.matmul(
                            ps[:, :GC], x_sb[tt // 4][:, co, 128 * (tt % 4):128 * (tt % 4 + 1)],
                            wrv_sb[:, co], start=(co == 0), stop=(co == CO - 1))
                    nc.gpsimd.tensor_copy(
                        out=v_aug[:, tt].rearrange("p (h e) -> p h e", e=D + 1)[:, :, :D],
                        in_=ps[:, :GC].rearrange("p (h d) -> p h d", d=D))

                # attention for query chunk ic=tcix, in query halves of 256
                # (4 key tiles of 256 i-cols share one 1024-wide span/exp)
                ic = tcix
                for ih in range(2):
                    njb = 4 * ic + 2 * ih + 2        # causal: last key tile 4ic+2ih+1
                    yac = [yap.tile([P, HLOC * (D + 1)], fp32, tag="ya",
                                    name=f"yac_{ic}_{ih}_{itl}") for itl in range(2)]
                    for hp in range(2):              # head pair = partition block
                        for u2 in range(2):
                            h = 2 * hp + u2
                            hb = 64 * u2
                            for s in range((njb + 3) // 4):
                                jbs = list(range(4 * s, min(4 * s + 4, njb)))
                                sp = spp.tile([P, 1024], fp32, tag="sp",
                                              name=f"sp_{ic}_{ih}_{h}_{s}")
                                pt = ptpool.tile([P, 1024], bf16, tag="pt",
                                                 name=f"pt_{ic}_{ih}_{h}_{s}")
                                qos = []
                                for c, jb in enumerate(jbs):
                                    m = jb - 4 * ic
                                    qo = max(0, 128 * m - 256 * ih)
                                    qos.append(qo)
                                    nc.tensor.matmul(
                                        sp[:, 256 * c + qo:256 * (c + 1)],
                                        qk_rope[2 + hp][hb:hb + 64, 128 * jb:128 * (jb + 1)],
                                        qk_rope[hp][hb:hb + 64,
                                                    512 * ic + 256 * ih + qo:
                                                    512 * ic + 256 * (ih + 1)],
                                        start=True, stop=True)
                                # exp over contiguous runs (qo>0 breaks a run)
                                runs, r0 = [], 0
                                for c, qo in enumerate(qos):
                                    if qo > 0:
                                        if 256 * c > r0:
                                            runs.append((r0, 256 * c))
                                        r0 = 256 * c + qo
                                if 256 * len(qos) > r0:
                                    runs.append((r0, 256 * len(qos)))
                                for (a, b) in runs:
                                    nc.scalar.activation(pt[:, a:b], sp[:, a:b],
                                                         EXP, scale=0.125)
                                for c, jb in enumerate(jbs):
                                    m = jb - 4 * ic
                                    for itl in range(2):
                                        it = 2 * ih + itl
                                        if it < m:
                                            continue
                                        if it == m:  # diagonal tile -> causal mask
                                            nc.vector.tensor_tensor(
                                                pt[:, 256 * c + 128 * itl:
                                                   256 * c + 128 * (itl + 1)],
                                                pt[:, 256 * c + 128 * itl:
                                                   256 * c + 128 * (itl + 1)],
                                                tri_sb[:], MUL)
                                        nc.tensor.matmul(
                                            yac[itl][:, 65 * h:65 * (h + 1)],
                                            pt[:, 256 * c + 128 * itl:
                                               256 * c + 128 * (itl + 1)],
                                            v_aug[:, jb, 65 * h:65 * (h + 1)],
                                            start=(jb == 0),
                                            stop=(jb == 4 * ic + it))
                    # normalize per 128-query tile; defer the transposes so
                    # their sp-slot turns don't stall the next half's spans
                    for itl in range(2):
                        it = 2 * ih + itl
                        tt = 4 * ic + it
                        yv = yac[itl].rearrange("p (h e) -> p h e", e=D + 1)
                        rec = work.tile([P, HLOC], fp32, tag="rec", name=f"rec_{tt}")
                        nc.vector.reciprocal(rec[:], yv[:, :, D])
                        yn = work.tile([P, GC], bf16, tag="yn", name=f"yn_{tt}",
                                       bufs=10)
                        nc.vector.tensor_tensor(
                            yn[:].rearrange("p (h d) -> p h d", d=D),
                            yv[:, :, :D], rec[:].broadcast_to((P, HLOC, D)), MUL)
                        for cb in range(2):
                            tp = yap.tile([P, P], bf16, tag="ya",
                                          name=f"tp_{tt}_{cb}")
                            nc.tensor.transpose(tp[:], yn[:, 128 * cb:128 * (cb + 1)],
                                                ident_sb[:])
                            nc.vector.tensor_copy(
                                out=ynT[cb][:, 128 * tt:128 * (tt + 1)], in_=tp[:])
            # output projection emitted last (= lowest scheduler priority):
            # its matmuls fill PE gaps in the ACT-paced attention stretches
            for tt in range(NTT):
                emit_proj(tt)


    _split_excess_waits(nc)
    return nc


def _split_excess_waits(nc, maxw=1):
    """Walrus codegen rejects instructions carrying >1 sem wait; move excess
    waits onto no-ops inserted immediately before, on the same engine."""
    import concourse.mybir as mybir
    n = 0
    for f in nc.m.functions:
        for bb in f.blocks:
            new = []
            for inst in bb.instructions:
                si = getattr(inst, "sync_info", None)
                if si is not None and si.on_wait and len(si.on_wait) > maxw:
                    waits = list(si.on_wait)
                    excess, keep = waits[:-maxw], waits[-maxw:]
                    for i in range(0, len(excess), maxw):
                        new.append(mybir.InstNoOp(
                            name=f"{inst.name}_wsp{n}_{i}", engine=inst.engine,
                            bass_nofuse=True,
                            sync_info=mybir.SyncInfo(on_wait=excess[i:i + maxw],
                                                     on_update=[])))
                    si.on_wait = keep
                    n += 1
                new.append(inst)
            bb.instructions[:] = new
    return n


def _get_runner():
    """Build the Bass program once and wrap it in a shard_map-jitted callable
    over the 8 cores (mirrors concourse.bass2jax.run_bass_via_pjrt)."""
    if "runner" in _CACHE:
        return _CACHE["runner"]
    import jax
    import numpy as _np
    from jax.sharding import Mesh, PartitionSpec
    from jax.experimental.shard_map import shard_map
    import concourse.mybir as mybir
    from concourse.bass2jax import _bass_exec_p, install_neuronx_cc_hook

    install_neuronx_cc_hook()
    from concourse.bass2jax import partition_id_tensor
    nc = _build_program()

    part_name = nc.partition_id_tensor.name if nc.partition_id_tensor else None
    in_names, out_names, out_avals = [], [], []
    for alloc in nc.m.functions[0].allocations:
        if not isinstance(alloc, mybir.MemoryLocationSet):
            continue
        name = alloc.memorylocations[0].name
        if alloc.kind == "ExternalInput":
            if name != part_name:
                in_names.append(name)
        elif alloc.kind == "ExternalOutput":
            out_names.append(name)
            out_avals.append(jax.core.ShapedArray(
                tuple(alloc.tensor_shape), mybir.dt.np(alloc.dtype)))
    n_params = len(in_names)
    all_names = in_names + out_names
    if part_name is not None:
        all_names = all_names + [part_name]

    def _body(*args):
        operands = list(args)
        if part_name is not None:
            operands.append(partition_id_tensor())
        outs = _bass_exec_p.bind(
            *operands, out_avals=tuple(out_avals), in_names=tuple(all_names),
            out_names=tuple(out_names), lowering_input_output_aliases=(),
            sim_require_finite=True, sim_require_nnan=True, nc=nc)
        return tuple(outs)

    devices = jax.devices()[:N_CORES]
    mesh = Mesh(_np.asarray(devices), ("core",))
    n_outs = len(out_names)
    sharded = jax.jit(
        shard_map(_body, mesh=mesh,
                  in_specs=(PartitionSpec("core"),) * (n_params + n_outs),
                  out_specs=(PartitionSpec("core"),) * n_outs,
                  check_rep=False),
        donate_argnums=tuple(range(n_params, n_params + n_outs)),
        keep_unused=True)

    runner = (sharded, in_names, out_names, out_avals)
    _CACHE["runner"] = runner
    return runner


def _prepare_core_inputs(x, w_qkv, w_proj):
    bf = ml_dtypes.bfloat16
    cosT, sinT = _CACHE.setdefault("rope", _rope_tables())
    # q_rope = q*cos + R(q * sinP) with sinP = half-swapped sin:
    #   (R(q*sinP))[d] = sign_d * q[s(d)] * sinP[s(d)] = rot_half(q)[d] * sin[d]
    sinP = np.concatenate([sinT[D // 2:D], sinT[:D // 2]], axis=0)
    sinP = np.concatenate([sinP, sinP], axis=0)[:P]
    cosT, sinT = cosT.astype(bf), sinP.astype(bf)
    # lhsT for the on-device rotate-half matmul: out = rmat.T @ q = R_pair @ q
    R = np.zeros((D, D), np.float32)
    for d in range(D // 2):
        R[d, d + D // 2] = -1.0
        R[d + D // 2, d] = 1.0
    R_pair = np.zeros((P, P), np.float32)
    R_pair[:D, :D] = R
    R_pair[D:, D:] = R
    rmat = np.ascontiguousarray(R_pair.T).astype(bf)
    ident = np.eye(P, dtype=np.float32).astype(bf)
    # tri[j, i'] = 1 if i' >= j  (keep keys at or before the query)
    tri = np.triu(np.ones((P, P), np.float32)).astype(bf)
    xTs = [np.ascontiguousarray(x[b].T).astype(bf) for b in range(B)]
    per_core = []
    for core in range(N_CORES):
        b, g = divmod(core, 4)
        rows = slice(GC * g, GC * (g + 1))
        wq = w_qkv[0 * C:1 * C][rows]
        wk = w_qkv[1 * C:2 * C][rows]
        wv = w_qkv[2 * C:3 * C][rows]
        wT = np.ascontiguousarray(
            np.concatenate([wq, wk, wv], axis=0).T).astype(bf)      # [C, 768]
        wpT = np.ascontiguousarray(w_proj[:, rows].T).astype(bf)    # [256, C]
        per_core.append({
            "xT": xTs[b], "wT": wT, "wpT": wpT, "rmat": rmat,
            "ident": ident, "cosT": cosT, "sinT": sinT, "tri": tri})
    return per_core


def _run_cores(per_core):
    from concourse import bass_utils
    if "nc" not in _CACHE:
        from concourse.bass2jax import install_neuronx_cc_hook
        install_neuronx_cc_hook()
        _CACHE["nc"] = _build_program()
    res = bass_utils.run_bass_kernel_spmd(
        _CACHE["nc"], per_core, core_ids=list(range(N_CORES)))
    return res.results


def kernel(x, w_qkv, w_proj):
    x = np.asarray(x, dtype=np.float32)
    w_qkv = np.asarray(w_qkv, dtype=np.float32)
    w_proj = np.asarray(w_proj, dtype=np.float32)
    per_core = _prepare_core_inputs(x, w_qkv, w_proj)
    results = _run_cores(per_core)
    out = np.zeros((B, T, C), dtype=np.float32)
    for core in range(N_CORES):
        b = core // 4
        out[b] += np.asarray(results[core]["out"], dtype=np.float32)
    return out


# revision 29
# speedup vs baseline: 1.3497x; 1.0538x over previous
"""Causal multi-head attention (RoPE) forward for Trainium2, 8 NeuronCores.

Problem: B=2, T=2048, C=1024, H=16, D=64.  out = proj(softmax(rope(q) rope(k)^T / 8, causal) @ v)

Sharding: 8 cores = 2 batches x 4 head-groups (4 heads each).
 - qkv projection column-sharded per head group, proj row-sharded; host sums
   the 4 per-group partial projections per batch (partials shipped as bf16).
 - Matmul cost on PE is proportional to OUTPUT free-size only, so the AV
   matmul runs in transposed layout (queries on partitions, 65 output
   columns per head): per 128-query tile and key tile,
     y_aug[i, 65h:65h+65] += pts[:, it]^T @ v_aug[:, jb, h]   (PSUM accum)
   with the ones column of v_aug producing the softmax denominator as a
   per-partition scalar -- normalization is then a single DVE multiply with
   a broadcast reciprocal, no rank-1 broadcast matmul needed.
 - Layouts:
     q^T,k^T   [d, t]  <- w^T-block as lhsT, x^T as rhs
     v         [t, d]  <- x^T-block as lhsT, w^T v-cols as rhs
     S^T       [j, i]  <- k^T-tile as lhsT, q^T as rhs       (j keys, i queries)
     P^T       = exp(S^T/8)  (no max subtraction: |scores| <= ~3 by construction)
     y_aug     [i, 65] <- P^T i-tile as lhsT, v_aug as rhs, PSUM-accumulated over j
     y^T       [c, i]  <- PE transpose of normalized y (bf16)
     out       [t, o]  <- y^T-tile as lhsT, wproj^T as rhs
 - RoPE: q_rope = q*cos + R(q*sinP), with sinP a half-swapped sin table and
   R the block-diagonal rotate-half matrix applied by one 128x128 PE matmul
   per tile.
 - Causality: key-tiles above the diagonal are skipped; on diagonal tiles
   QK/exp compute only the surviving query range and a single triangular
   [128,128] mask is applied after the exp; AV skips dead (i-tile, j-tile)
   pairs entirely.
 - Single fused chunk pipeline: qkv(tc) -> rope(tc) -> attention(ic=tc) with
   the output projection deferred to fill tail PE gaps.
"""

import numpy as np
import ml_dtypes

_CACHE = {}

B, T, C = 2, 2048, 1024
HLOC, D = 4, 64            # heads per core, head dim
GC = HLOC * D              # 256 channels per group
P = 128
NTT = T // P               # 16 key/row tiles
NIC = T // 512             # 4 query chunks of 512
THETA = 10000.0
N_CORES = 8


def _rope_tables():
    freqs = 1.0 / THETA ** (np.arange(0, D, 2, dtype=np.float32) / D)
    t = np.arange(T, dtype=np.float32)
    f = np.outer(t, freqs)                          # [T, 32]
    emb = np.concatenate([f, f], axis=-1)           # [T, 64]
    cosT = np.cos(emb).T.astype(np.float32)         # [64, T]
    sinT = np.sin(emb).T.astype(np.float32)
    # tile to 128 partitions (2 heads per partition block)
    return (np.concatenate([cosT, cosT], 0), np.concatenate([sinT, sinT], 0))


def _build_program():
    import concourse.bass as bass
    import concourse.mybir as mybir
    import concourse.tile as tile

    dt = mybir.dt
    fp32 = dt.float32
    bf16 = dt.bfloat16
    EXP = mybir.ActivationFunctionType.Exp
    MUL = mybir.AluOpType.mult

    nc = bass.Bass("TRN2", target_bir_lowering=False, debug=False,
                   enable_asserts=True, num_devices=N_CORES)

    xT = nc.dram_tensor("xT", [C, T], bf16, kind="ExternalInput").ap()
    wT = nc.dram_tensor("wT", [C, 3 * GC], bf16, kind="ExternalInput").ap()        # q,k,v col-blocks
    rmat_d = nc.dram_tensor("rmat", [P, P], bf16, kind="ExternalInput").ap()
    ident_d = nc.dram_tensor("ident", [P, P], bf16, kind="ExternalInput").ap()
    wpT = nc.dram_tensor("wpT", [GC, C], bf16, kind="ExternalInput").ap()
    cosT_d = nc.dram_tensor("cosT", [P, T], bf16, kind="ExternalInput").ap()
    sinT_d = nc.dram_tensor("sinT", [P, T], bf16, kind="ExternalInput").ap()
    tri_d = nc.dram_tensor("tri", [P, P], bf16, kind="ExternalInput").ap()
    out_d = nc.dram_tensor("out", [T, C], bf16, kind="ExternalOutput").ap()

    CO = C // P  # 8 contraction blocks

    with tile.TileContext(nc) as tc:
        with (
            tc.tile_pool(name="persist", bufs=1) as persist,
            tc.tile_pool(name="work", bufs=8) as work,
            tc.tile_pool(name="pt", bufs=6) as ptpool,
            tc.tile_pool(name="outp", bufs=8) as outpool,
            tc.tile_pool(name="wk", bufs=2, space="PSUM") as wkp,
            tc.tile_pool(name="sp", bufs=2, space="PSUM") as spp,
            tc.tile_pool(name="ya", bufs=2, space="PSUM") as yap,
        ):
            # ---- persistent SBUF loads -------------------------------------
            # Few big strided DMAs spread across HWDGE queues (SP + ACT + DVE)
            # so tiles land in first-use order without per-tile queue overhead.
            warm_w = persist.tile([P, P], bf16, tag="warm_w")
            nc.gpsimd.memset(warm_w[:], 0.0)
            warm = wkp.tile([P, P], fp32, tag="wk", name="warmup")
            for i in range(56):
                nc.tensor.matmul(warm[:], warm_w[:], warm_w[:],
                                 start=True, stop=True, skip_group_check=True)
            # All input DMAs on the SP queue in need-order: transfers serialize
            # on the DMA engines, so queue order == arrival order.
            rmat_sb = persist.tile([P, P], bf16, tag="rmat")
            nc.sync.dma_start(rmat_sb[:], rmat_d[:])
            x_sb = [persist.tile([P, CO, 512], bf16, tag=f"x{i}", name=f"x{i}")
                    for i in range(NIC)]
            def load_x(tcix, half):
                co0, co1 = (0, 4) if half == 0 else (4, CO)
                src = xT[P * co0:P * co1, 512 * tcix:512 * (tcix + 1)].rearrange(
                    "(co p) t -> p co t", p=P)
                nc.sync.dma_start(x_sb[tcix][:, co0:co1], src)
            wqk_sb = persist.tile([P, CO, 512], bf16, tag="wqk")
            nc.sync.dma_start(wqk_sb[:, 0:4],
                              wT[0:512, 0:512].rearrange("(co p) q -> p co q", p=P))
            load_x(0, 0)
            nc.sync.dma_start(wqk_sb[:, 4:8],
                              wT[512:1024, 0:512].rearrange("(co p) q -> p co q", p=P))
            load_x(0, 1)
            cos_sb = persist.tile([P, T], bf16, tag="cos")
            nc.sync.dma_start(cos_sb[:], cosT_d[:])
            sin_sb = persist.tile([P, T], bf16, tag="sin")
            nc.sync.dma_start(sin_sb[:], sinT_d[:])
            wrv_sb = persist.tile([P, CO, 256], bf16, tag="wrv")
            nc.sync.dma_start(wrv_sb[:], wT[:, 512:768].rearrange("(co p) q -> p co q", p=P))
            tri_sb = persist.tile([P, P], bf16, tag="tri")
            nc.sync.dma_start(tri_sb[:], tri_d[:])
            ident_sb = persist.tile([P, P], bf16, tag="ident")
            nc.sync.dma_start(ident_sb[:], ident_d[:])
            load_x(1, 0)
            load_x(1, 1)
            wpT_sb = persist.tile([P, 2, C], bf16, tag="wpT")
            nc.sync.dma_start(wpT_sb[:], wpT.rearrange("(cb p) o -> p cb o", p=P))
            load_x(2, 0)
            load_x(2, 1)
            load_x(3, 0)
            load_x(3, 1)

            # rope outputs: q^T,k^T per 2-head block [128, T] bf16
            qk_rope = [persist.tile([P, T], bf16, tag=f"qkrope{i}", name=f"qkrope{i}") for i in range(4)]
            # v with ones column per head: [128part=t, 16 ttiles, 4*65]
            v_aug = persist.tile([P, NTT, HLOC * (D + 1)], bf16, tag="vaug")
            nc.vector.memset(v_aug[:], 1.0)
            # normalized y^T blocks (2 channel blocks of 128) [128, T] bf16
            ynT = [persist.tile([P, T], bf16, tag=f"ynT{i}", name=f"ynT{i}") for i in range(2)]

            def emit_proj(tt):
                for oc in range(2):
                    ps = wkp.tile([P, 512], fp32, tag="wk", name=f"pso_{tt}_{oc}")
                    for cb in range(2):
                        nc.tensor.matmul(
                            ps[:], ynT[cb][:, 128 * tt:128 * (tt + 1)],
                            wpT_sb[:, cb, 512 * oc:512 * (oc + 1)],
                            start=(cb == 0), stop=(cb == 1))
                    ob = outpool.tile([P, 512], bf16, tag="ob")
                    nc.gpsimd.tensor_copy(out=ob[:], in_=ps[:])
                    nc.sync.dma_start(out_d[128 * tt:128 * (tt + 1), 512 * oc:512 * (oc + 1)], ob[:])

            # ---- fused pipeline: per 512-token chunk tc:
            #   qkv(tc) -> rope(tc) -> attention(ic=tc, keys<=tc)
            # (causality means chunk tc's attention needs only k/v chunks <= tc,
            #  so the ACT-bound softmax overlaps the PE-bound qkv of later chunks)
            for tcix in range(NIC):
                # q/k projections (ft 0,1 q; 2,3 k) + rotated partners, + rope
                for ft in range(4):
                    ps = wkp.tile([P, 512], fp32, tag="wk", name=f"psq_{ft}_{tcix}")
                    for co in range(CO):
                        nc.tensor.matmul(
                            ps[:], wqk_sb[:, co, 128 * ft:128 * (ft + 1)],
                            x_sb[tcix][:, co], start=(co == 0), stop=(co == CO - 1))
                    t1 = work.tile([P, 512], bf16, tag="t1")
                    nc.vector.tensor_tensor(t1[:], ps[:], cos_sb[:, 512 * tcix:512 * (tcix + 1)], MUL)
                    u = work.tile([P, 512], bf16, tag="u")
                    nc.vector.tensor_tensor(u[:], ps[:], sin_sb[:, 512 * tcix:512 * (tcix + 1)], MUL)
                    psr = wkp.tile([P, 512], fp32, tag="wk", name=f"psr_{ft}_{tcix}")
                    nc.tensor.matmul(psr[:], rmat_sb[:], u[:], start=True, stop=True)
                    nc.vector.tensor_add(qk_rope[ft][:, 512 * tcix:512 * (tcix + 1)], psr[:], t1[:])
                # v for this chunk's 4 key tiles
                for tt in range(4 * tcix, 4 * tcix + 4):
                    ps = wkp.tile([P, 512], fp32, tag="wk", name=f"psv_{tt}")
                    for co in range(CO):
                        nc.tensor.matmul(
                            ps[:, :GC], x_sb[tt // 4][:, co, 128 * (tt % 4):128 * (tt % 4 + 1)],
                            wrv_sb[:, co], start=(co == 0), stop=(co == CO - 1))
                    nc.gpsimd.tensor_copy(
                        out=v_aug[:, tt].rearrange("p (h e) -> p h e", e=D + 1)[:, :, :D],
                        in_=ps[:, :GC].rearrange("p (h d) -> p h d", d=D))

                # attention for query chunk ic=tcix, in query halves of 256
                # (4 key tiles of 256 i-cols share one 1024-wide span/exp)
                ic = tcix
                for ih in range(2):
                    njb = 4 * ic + 2 * ih + 2        # causal: last key tile 4ic+2ih+1
                    yac = [yap.tile([P, HLOC * (D + 1)], fp32, tag="ya",
                                    name=f"yac_{ic}_{ih}_{itl}") for itl in range(2)]
                    for hp in range(2):              # head pair = partition block
                        for u2 in range(2):
                            h = 2 * hp + u2
                            hb = 64 * u2
                            for s in range((njb + 3) // 4):
                                jbs = list(range(4 * s, min(4 * s + 4, njb)))
                                sp = spp.tile([P, 1024], fp32, tag="sp",
                                              name=f"sp_{ic}_{ih}_{h}_{s}")
                                pt = ptpool.tile([P, 1024], bf16, tag="pt",
                                                 name=f"pt_{ic}_{ih}_{h}_{s}")
                                qos = []
                                for c, jb in enumerate(jbs):
                                    m = jb - 4 * ic
                                    qo = max(0, 128 * m - 256 * ih)
                                    qos.append(qo)
                                    nc.tensor.matmul(
                                        sp[:, 256 * c + qo:256 * (c + 1)],
                                        qk_rope[2 + hp][hb:hb + 64, 128 * jb:128 * (jb + 1)],
                                        qk_rope[hp][hb:hb + 64,
                                                    512 * ic + 256 * ih + qo:
                                                    512 * ic + 256 * (ih + 1)],
                                        start=True, stop=True)
                                # exp over contiguous runs (qo>0 breaks a run)
                                runs, r0 = [], 0
                                for c, qo in enumerate(qos):
                                    if qo > 0:
                                        if 256 * c > r0:
                                            runs.append((r0, 256 * c))
                                        r0 = 256 * c + qo
                                if 256 * len(qos) > r0:
                                    runs.append((r0, 256 * len(qos)))
                                for (a, b) in runs:
                                    nc.scalar.activation(pt[:, a:b], sp[:, a:b],
                                                         EXP, scale=0.125)
                                for c, jb in enumerate(jbs):
                                    m = jb - 4 * ic
                                    for itl in range(2):
                                        it = 2 * ih + itl
                                        if it < m:
                                            continue
                                        if it == m:  # diagonal tile -> causal mask
                                            nc.vector.tensor_tensor(
                                                pt[:, 256 * c + 128 * itl:
                                                   256 * c + 128 * (itl + 1)],
                                                pt[:, 256 * c + 128 * itl:
                                                   256 * c + 128 * (itl + 1)],
                                                tri_sb[:], MUL)
                                        nc.tensor.matmul(
                                            yac[itl][:, 65 * h:65 * (h + 1)],
                                            pt[:, 256 * c + 128 * itl:
                                               256 * c + 128 * (itl + 1)],
                                            v_aug[:, jb, 65 * h:65 * (h + 1)],
                                            start=(jb == 0),
                                            stop=(jb == 4 * ic + it))
                    # normalize per 128-query tile; defer the transposes so
                    # their sp-slot turns don't stall the next half's spans
                    for itl in range(2):
                        it = 2 * ih + itl
                        tt = 4 * ic + it
                        yv = yac[itl].rearrange("p (h e) -> p h e", e=D + 1)
                        rec = work.tile([P, HLOC], fp32, tag="rec", name=f"rec_{tt}")
                        nc.vector.reciprocal(rec[:], yv[:, :, D])
                        yn = work.tile([P, GC], bf16, tag="yn", name=f"yn_{tt}",
                                       bufs=10)
                        nc.vector.tensor_tensor(
                            yn[:].rearrange("p (h d) -> p h d", d=D),
                            yv[:, :, :D], rec[:].broadcast_to((P, HLOC, D)), MUL)
                        for cb in range(2):
                            tp = yap.tile([P, P], bf16, tag="ya",
                                          name=f"tp_{tt}_{cb}")
                            nc.tensor.transpose(tp[:], yn[:, 128 * cb:128 * (cb + 1)],
                                                ident_sb[:])
                            nc.vector.tensor_copy(
                                out=ynT[cb][:, 128 * tt:128 * (tt + 1)], in_=tp[:])
            # output projection emitted last (= lowest scheduler priority):
            # its matmuls fill PE gaps in the ACT-paced attention stretches
            for tt in range(NTT):
                emit_proj(tt)


    _split_excess_waits(nc)
    return nc


def _split_excess_waits(nc, maxw=1):
    """Walrus codegen rejects instructions carrying >1 sem wait; move excess
    waits onto no-ops inserted immediately before, on the same engine."""
    import concourse.mybir as mybir
    n = 0
    for f in nc.m.functions:
        for bb in f.blocks:
            new = []
            for inst in bb.instructions:
                si = getattr(inst, "sync_info", None)
                if si is not None and si.on_wait and len(si.on_wait) > maxw:
                    waits = list(si.on_wait)
                    excess, keep = waits[:-maxw], waits[-maxw:]
                    for i in range(0, len(excess), maxw):
                        new.append(mybir.InstNoOp(
                            name=f"{inst.name}_wsp{n}_{i}", engine=inst.engine,
                            bass_nofuse=True,
                            sync_info=mybir.SyncInfo(on_wait=excess[i:i + maxw],
                                                     on_update=[])))
                    si.on_wait = keep
                    n += 1
                new.append(inst)
            bb.instructions[:] = new
    return n


def _get_runner():
    """Build the Bass program once and wrap it in a shard_map-jitted callable
    over the 8 cores (mirrors concourse.bass2jax.run_bass_via_pjrt)."""
    if "runner" in _CACHE:
        return _CACHE["runner"]
    import jax
    import numpy as _np
    from jax.sharding import Mesh, PartitionSpec
    from jax.experimental.shard_map import shard_map
    import concourse.mybir as mybir
    from concourse.bass2jax import _bass_exec_p, install_neuronx_cc_hook

    install_neuronx_cc_hook()
    from concourse.bass2jax import partition_id_tensor
    nc = _build_program()

    part_name = nc.partition_id_tensor.name if nc.partition_id_tensor else None
    in_names, out_names, out_avals = [], [], []
    for alloc in nc.m.functions[0].allocations:
        if not isinstance(alloc, mybir.MemoryLocationSet):
            continue
        name = alloc.memorylocations[0].name
        if alloc.kind == "ExternalInput":
            if name != part_name:
                in_names.append(name)
        elif alloc.kind == "ExternalOutput":
            out_names.append(name)
            out_avals.append(jax.core.ShapedArray(
                tuple(alloc.tensor_shape), mybir.dt.np(alloc.dtype)))
    n_params = len(in_names)
    all_names = in_names + out_names
    if part_name is not None:
        all_names = all_names + [part_name]

    def _body(*args):
        operands = list(args)
        if part_name is not None:
            operands.append(partition_id_tensor())
        outs = _bass_exec_p.bind(
            *operands, out_avals=tuple(out_avals), in_names=tuple(all_names),
            out_names=tuple(out_names), lowering_input_output_aliases=(),
            sim_require_finite=True, sim_require_nnan=True, nc=nc)
        return tuple(outs)

    devices = jax.devices()[:N_CORES]
    mesh = Mesh(_np.asarray(devices), ("core",))
    n_outs = len(out_names)
    sharded = jax.jit(
        shard_map(_body, mesh=mesh,
                  in_specs=(PartitionSpec("core"),) * (n_params + n_outs),
                  out_specs=(PartitionSpec("core"),) * n_outs,
                  check_rep=False),
        donate_argnums=tuple(range(n_params, n_params + n_outs)),
        keep_unused=True)

    runner = (sharded, in_names, out_names, out_avals)
    _CACHE["runner"] = runner
    return runner


def _prepare_core_inputs(x, w_qkv, w_proj):
    bf = ml_dtypes.bfloat16
    cosT, sinT = _CACHE.setdefault("rope", _rope_tables())
    # q_rope = q*cos + R(q * sinP) with sinP = half-swapped sin:
    #   (R(q*sinP))[d] = sign_d * q[s(d)] * sinP[s(d)] = rot_half(q)[d] * sin[d]
    sinP = np.concatenate([sinT[D // 2:D], sinT[:D // 2]], axis=0)
    sinP = np.concatenate([sinP, sinP], axis=0)[:P]
    cosT, sinT = cosT.astype(bf), sinP.astype(bf)
    # lhsT for the on-device rotate-half matmul: out = rmat.T @ q = R_pair @ q
    R = np.zeros((D, D), np.float32)
    for d in range(D // 2):
        R[d, d + D // 2] = -1.0
        R[d + D // 2, d] = 1.0
    R_pair = np.zeros((P, P), np.float32)
    R_pair[:D, :D] = R
    R_pair[D:, D:] = R
    rmat = np.ascontiguousarray(R_pair.T).astype(bf)
    ident = np.eye(P, dtype=np.float32).astype(bf)
    # tri[j, i'] = 1 if i' >= j  (keep keys at or before the query)
    tri = np.triu(np.ones((P, P), np.float32)).astype(bf)
    xTs = [np.ascontiguousarray(x[b].T).astype(bf) for b in range(B)]
    per_core = []
    for core in range(N_CORES):
        b, g = divmod(core, 4)
        rows = slice(GC * g, GC * (g + 1))
        wq = w_qkv[0 * C:1 * C][rows]
        wk = w_qkv[1 * C:2 * C][rows]
        wv = w_qkv[2 * C:3 * C][rows]
        wT = np.ascontiguousarray(
            np.concatenate([wq, wk, wv], axis=0).T).astype(bf)      # [C, 768]
        wpT = np.ascontiguousarray(w_proj[:, rows].T).astype(bf)    # [256, C]
        per_core.append({
            "xT": xTs[b], "wT": wT, "wpT": wpT, "rmat": rmat,
            "ident": ident, "cosT": cosT, "sinT": sinT, "tri": tri})
    return per_core


def _run_cores(per_core):
    from concourse import bass_utils
    if "nc" not in _CACHE:
        from concourse.bass2jax import install_neuronx_cc_hook
        install_neuronx_cc_hook()
        _CACHE["nc"] = _build_program()
    res = bass_utils.run_bass_kernel_spmd(
        _CACHE["nc"], per_core, core_ids=list(range(N_CORES)))
    return res.results


def kernel(x, w_qkv, w_proj):
    x = np.asarray(x, dtype=np.float32)
    w_qkv = np.asarray(w_qkv, dtype=np.float32)
    w_proj = np.asarray(w_proj, dtype=np.float32)
    per_core = _prepare_core_inputs(x, w_qkv, w_proj)
    results = _run_cores(per_core)
    out = np.zeros((B, T, C), dtype=np.float32)
    for core in range(N_CORES):
        b = core // 4
        out[b] += np.asarray(results[core]["out"], dtype=np.float32)
    return out


# revision 31
# speedup vs baseline: 1.3644x; 1.0109x over previous
"""Causal multi-head attention (RoPE) forward for Trainium2, 8 NeuronCores.

Problem: B=2, T=2048, C=1024, H=16, D=64.  out = proj(softmax(rope(q) rope(k)^T / 8, causal) @ v)

Sharding: 8 cores = 2 batches x 4 head-groups (4 heads each).
 - qkv projection column-sharded per head group, proj row-sharded; host sums
   the 4 per-group partial projections per batch (partials shipped as bf16).
 - Matmul cost on PE is proportional to OUTPUT free-size only, so the AV
   matmul runs in transposed layout (queries on partitions, 65 output
   columns per head): per 128-query tile and key tile,
     y_aug[i, 65h:65h+65] += pts[:, it]^T @ v_aug[:, jb, h]   (PSUM accum)
   with the ones column of v_aug producing the softmax denominator as a
   per-partition scalar -- normalization is then a single DVE multiply with
   a broadcast reciprocal, no rank-1 broadcast matmul needed.
 - Layouts:
     q^T,k^T   [d, t]  <- w^T-block as lhsT, x^T as rhs
     v         [t, d]  <- x^T-block as lhsT, w^T v-cols as rhs
     S^T       [j, i]  <- k^T-tile as lhsT, q^T as rhs       (j keys, i queries)
     P^T       = exp(S^T/8)  (no max subtraction: |scores| <= ~3 by construction)
     y_aug     [i, 65] <- P^T i-tile as lhsT, v_aug as rhs, PSUM-accumulated over j
     y^T       [c, i]  <- PE transpose of normalized y (bf16)
     out       [t, o]  <- y^T-tile as lhsT, wproj^T as rhs
 - RoPE: q_rope = q*cos + R(q*sinP), with sinP a half-swapped sin table and
   R the block-diagonal rotate-half matrix applied by one 128x128 PE matmul
   per tile.
 - Causality: key-tiles above the diagonal are skipped; on diagonal tiles
   QK/exp compute only the surviving query range and a single triangular
   [128,128] mask is applied after the exp; AV skips dead (i-tile, j-tile)
   pairs entirely.
 - Single fused chunk pipeline: qkv(tc) -> rope(tc) -> attention(ic=tc) with
   the output projection deferred to fill tail PE gaps.
"""

import numpy as np
import ml_dtypes

_CACHE = {}

B, T, C = 2, 2048, 1024
HLOC, D = 4, 64            # heads per core, head dim
GC = HLOC * D              # 256 channels per group
P = 128
NTT = T // P               # 16 key/row tiles
NIC = T // 512             # 4 query chunks of 512
THETA = 10000.0
N_CORES = 8


def _rope_tables():
    freqs = 1.0 / THETA ** (np.arange(0, D, 2, dtype=np.float32) / D)
    t = np.arange(T, dtype=np.float32)
    f = np.outer(t, freqs)                          # [T, 32]
    emb = np.concatenate([f, f], axis=-1)           # [T, 64]
    cosT = np.cos(emb).T.astype(np.float32)         # [64, T]
    sinT = np.sin(emb).T.astype(np.float32)
    # tile to 128 partitions (2 heads per partition block)
    return (np.concatenate([cosT, cosT], 0), np.concatenate([sinT, sinT], 0))


def _build_program():
    import concourse.bass as bass
    import concourse.mybir as mybir
    import concourse.tile as tile

    dt = mybir.dt
    fp32 = dt.float32
    bf16 = dt.bfloat16
    EXP = mybir.ActivationFunctionType.Exp
    MUL = mybir.AluOpType.mult

    nc = bass.Bass("TRN2", target_bir_lowering=False, debug=False,
                   enable_asserts=True, num_devices=N_CORES)

    xT = nc.dram_tensor("xT", [C, T], bf16, kind="ExternalInput").ap()
    wT = nc.dram_tensor("wT", [C, 3 * GC], bf16, kind="ExternalInput").ap()        # q,k,v col-blocks
    rmat_d = nc.dram_tensor("rmat", [P, P], bf16, kind="ExternalInput").ap()
    ident_d = nc.dram_tensor("ident", [P, P], bf16, kind="ExternalInput").ap()
    wpT = nc.dram_tensor("wpT", [GC, C], bf16, kind="ExternalInput").ap()
    cosT_d = nc.dram_tensor("cosT", [P, T], bf16, kind="ExternalInput").ap()
    sinT_d = nc.dram_tensor("sinT", [P, T], bf16, kind="ExternalInput").ap()
    tri_d = nc.dram_tensor("tri", [P, P], bf16, kind="ExternalInput").ap()
    out_d = nc.dram_tensor("out", [T, C], bf16, kind="ExternalOutput").ap()

    CO = C // P  # 8 contraction blocks

    with tile.TileContext(nc) as tc:
        with (
            tc.tile_pool(name="persist", bufs=1) as persist,
            tc.tile_pool(name="work", bufs=8) as work,
            tc.tile_pool(name="pt", bufs=6) as ptpool,
            tc.tile_pool(name="outp", bufs=8) as outpool,
            tc.tile_pool(name="wk", bufs=2, space="PSUM") as wkp,
            tc.tile_pool(name="sp", bufs=2, space="PSUM") as spp,
            tc.tile_pool(name="ya", bufs=2, space="PSUM") as yap,
        ):
            # ---- persistent SBUF loads -------------------------------------
            # Few big strided DMAs spread across HWDGE queues (SP + ACT + DVE)
            # so tiles land in first-use order without per-tile queue overhead.
            warm_w = persist.tile([P, P], bf16, tag="warm_w")
            nc.gpsimd.memset(warm_w[:], 0.0)
            warm = wkp.tile([P, P], fp32, tag="wk", name="warmup")
            for i in range(56):
                nc.tensor.matmul(warm[:], warm_w[:], warm_w[:],
                                 start=True, stop=True, skip_group_check=True)
            # All input DMAs on the SP queue in need-order: transfers serialize
            # on the DMA engines, so queue order == arrival order.
            rmat_sb = persist.tile([P, P], bf16, tag="rmat")
            nc.sync.dma_start(rmat_sb[:], rmat_d[:])
            x_sb = [persist.tile([P, CO, 512], bf16, tag=f"x{i}", name=f"x{i}")
                    for i in range(NIC)]
            def load_x(tcix, half):
                co0, co1 = (0, 4) if half == 0 else (4, CO)
                src = xT[P * co0:P * co1, 512 * tcix:512 * (tcix + 1)].rearrange(
                    "(co p) t -> p co t", p=P)
                nc.sync.dma_start(x_sb[tcix][:, co0:co1], src)
            wqk_sb = persist.tile([P, CO, 512], bf16, tag="wqk")
            nc.sync.dma_start(wqk_sb[:, 0:4],
                              wT[0:512, 0:512].rearrange("(co p) q -> p co q", p=P))
            load_x(0, 0)
            nc.sync.dma_start(wqk_sb[:, 4:8],
                              wT[512:1024, 0:512].rearrange("(co p) q -> p co q", p=P))
            load_x(0, 1)
            cos_sb = persist.tile([P, T], bf16, tag="cos")
            nc.sync.dma_start(cos_sb[:], cosT_d[:])
            sin_sb = persist.tile([P, T], bf16, tag="sin")
            nc.sync.dma_start(sin_sb[:], sinT_d[:])
            wrv_sb = persist.tile([P, CO, 256], bf16, tag="wrv")
            nc.sync.dma_start(wrv_sb[:], wT[:, 512:768].rearrange("(co p) q -> p co q", p=P))
            tri_sb = persist.tile([P, P], bf16, tag="tri")
            nc.sync.dma_start(tri_sb[:], tri_d[:])
            ident_sb = persist.tile([P, P], bf16, tag="ident")
            nc.sync.dma_start(ident_sb[:], ident_d[:])
            load_x(1, 0)
            load_x(1, 1)
            wpT_sb = persist.tile([P, 2, C], bf16, tag="wpT")
            nc.sync.dma_start(wpT_sb[:], wpT.rearrange("(cb p) o -> p cb o", p=P))
            load_x(2, 0)
            load_x(2, 1)
            load_x(3, 0)
            load_x(3, 1)

            # rope outputs: q^T,k^T per 2-head block [128, T] bf16
            qk_rope = [persist.tile([P, T], bf16, tag=f"qkrope{i}", name=f"qkrope{i}") for i in range(4)]
            # v with ones column per head: [128part=t, 16 ttiles, 4*65]
            v_aug = persist.tile([P, NTT, HLOC * (D + 1)], bf16, tag="vaug")
            nc.vector.memset(v_aug[:], 1.0)
            # normalized y^T blocks (2 channel blocks of 128) [128, T] bf16
            ynT = [persist.tile([P, T], bf16, tag=f"ynT{i}", name=f"ynT{i}") for i in range(2)]

            def emit_proj(tt):
                for oc in range(2):
                    ps = wkp.tile([P, 512], fp32, tag="wk", name=f"pso_{tt}_{oc}")
                    for cb in range(2):
                        nc.tensor.matmul(
                            ps[:], ynT[cb][:, 128 * tt:128 * (tt + 1)],
                            wpT_sb[:, cb, 512 * oc:512 * (oc + 1)],
                            start=(cb == 0), stop=(cb == 1))
                    ob = outpool.tile([P, 512], bf16, tag="ob")
                    nc.vector.tensor_copy(out=ob[:], in_=ps[:])
                    nc.sync.dma_start(out_d[128 * tt:128 * (tt + 1), 512 * oc:512 * (oc + 1)], ob[:])

            # ---- fused pipeline: per 512-token chunk tc:
            #   qkv(tc) -> rope(tc) -> attention(ic=tc, keys<=tc)
            # (causality means chunk tc's attention needs only k/v chunks <= tc,
            #  so the ACT-bound softmax overlaps the PE-bound qkv of later chunks)
            for tcix in range(NIC):
                # q/k projections (ft 0,1 q; 2,3 k) + rotated partners, + rope
                for ft in range(4):
                    ps = wkp.tile([P, 512], fp32, tag="wk", name=f"psq_{ft}_{tcix}")
                    for co in range(CO):
                        nc.tensor.matmul(
                            ps[:], wqk_sb[:, co, 128 * ft:128 * (ft + 1)],
                            x_sb[tcix][:, co], start=(co == 0), stop=(co == CO - 1))
                    t1 = work.tile([P, 512], bf16, tag="t1")
                    nc.vector.tensor_tensor(t1[:], ps[:], cos_sb[:, 512 * tcix:512 * (tcix + 1)], MUL)
                    u = work.tile([P, 512], bf16, tag="u")
                    nc.vector.tensor_tensor(u[:], ps[:], sin_sb[:, 512 * tcix:512 * (tcix + 1)], MUL)
                    psr = wkp.tile([P, 512], fp32, tag="wk", name=f"psr_{ft}_{tcix}")
                    nc.tensor.matmul(psr[:], rmat_sb[:], u[:], start=True, stop=True)
                    nc.vector.tensor_add(qk_rope[ft][:, 512 * tcix:512 * (tcix + 1)], psr[:], t1[:])
                # v for this chunk's 4 key tiles
                for tt in range(4 * tcix, 4 * tcix + 4):
                    ps = wkp.tile([P, 512], fp32, tag="wk", name=f"psv_{tt}")
                    for co in range(CO):
                        nc.tensor.matmul(
                            ps[:, :GC], x_sb[tt // 4][:, co, 128 * (tt % 4):128 * (tt % 4 + 1)],
                            wrv_sb[:, co], start=(co == 0), stop=(co == CO - 1))
                    nc.vector.tensor_copy(
                        out=v_aug[:, tt].rearrange("p (h e) -> p h e", e=D + 1)[:, :, :D],
                        in_=ps[:, :GC].rearrange("p (h d) -> p h d", d=D))

                # attention for query chunk ic=tcix, in query halves of 256
                # (4 key tiles of 256 i-cols share one 1024-wide span/exp)
                ic = tcix
                for ih in range(2):
                    njb = 4 * ic + 2 * ih + 2        # causal: last key tile 4ic+2ih+1
                    yac = [yap.tile([P, HLOC * (D + 1)], fp32, tag="ya",
                                    name=f"yac_{ic}_{ih}_{itl}") for itl in range(2)]
                    for hp in range(2):              # head pair = partition block
                        for u2 in range(2):
                            h = 2 * hp + u2
                            hb = 64 * u2
                            for s in range((njb + 3) // 4):
                                jbs = list(range(4 * s, min(4 * s + 4, njb)))
                                sp = spp.tile([P, 1024], fp32, tag="sp",
                                              name=f"sp_{ic}_{ih}_{h}_{s}")
                                pt = ptpool.tile([P, 1024], bf16, tag="pt",
                                                 name=f"pt_{ic}_{ih}_{h}_{s}")
                                qos = []
                                for c, jb in enumerate(jbs):
                                    m = jb - 4 * ic
                                    qo = max(0, 128 * m - 256 * ih)
                                    qos.append(qo)
                                    nc.tensor.matmul(
                                        sp[:, 256 * c + qo:256 * (c + 1)],
                                        qk_rope[2 + hp][hb:hb + 64, 128 * jb:128 * (jb + 1)],
                                        qk_rope[hp][hb:hb + 64,
                                                    512 * ic + 256 * ih + qo:
                                                    512 * ic + 256 * (ih + 1)],
                                        start=True, stop=True)
                                # exp over contiguous runs (qo>0 breaks a run)
                                runs, r0 = [], 0
                                for c, qo in enumerate(qos):
                                    if qo > 0:
                                        if 256 * c > r0:
                                            runs.append((r0, 256 * c))
                                        r0 = 256 * c + qo
                                if 256 * len(qos) > r0:
                                    runs.append((r0, 256 * len(qos)))
                                for (a, b) in runs:
                                    nc.scalar.activation(pt[:, a:b], sp[:, a:b],
                                                         EXP, scale=0.125)
                                for c, jb in enumerate(jbs):
                                    m = jb - 4 * ic
                                    for itl in range(2):
                                        it = 2 * ih + itl
                                        if it < m:
                                            continue
                                        if it == m:  # diagonal tile -> causal mask
                                            nc.vector.tensor_tensor(
                                                pt[:, 256 * c + 128 * itl:
                                                   256 * c + 128 * (itl + 1)],
                                                pt[:, 256 * c + 128 * itl:
                                                   256 * c + 128 * (itl + 1)],
                                                tri_sb[:], MUL)
                                        nc.tensor.matmul(
                                            yac[itl][:, 65 * h:65 * (h + 1)],
                                            pt[:, 256 * c + 128 * itl:
                                               256 * c + 128 * (itl + 1)],
                                            v_aug[:, jb, 65 * h:65 * (h + 1)],
                                            start=(jb == 0),
                                            stop=(jb == 4 * ic + it))
                    # normalize per 128-query tile; defer the transposes so
                    # their sp-slot turns don't stall the next half's spans
                    for itl in range(2):
                        it = 2 * ih + itl
                        tt = 4 * ic + it
                        yv = yac[itl].rearrange("p (h e) -> p h e", e=D + 1)
                        rec = work.tile([P, HLOC], fp32, tag="rec", name=f"rec_{tt}")
                        nc.vector.reciprocal(rec[:], yv[:, :, D])
                        yn = work.tile([P, GC], bf16, tag="yn", name=f"yn_{tt}",
                                       bufs=10)
                        nc.vector.tensor_tensor(
                            yn[:].rearrange("p (h d) -> p h d", d=D),
                            yv[:, :, :D], rec[:].broadcast_to((P, HLOC, D)), MUL)
                        for cb in range(2):
                            tp = yap.tile([P, P], bf16, tag="ya",
                                          name=f"tp_{tt}_{cb}")
                            nc.tensor.transpose(tp[:], yn[:, 128 * cb:128 * (cb + 1)],
                                                ident_sb[:])
                            nc.vector.tensor_copy(
                                out=ynT[cb][:, 128 * tt:128 * (tt + 1)], in_=tp[:])
            # output projection emitted last (= lowest scheduler priority):
            # its matmuls fill PE gaps in the ACT-paced attention stretches
            for tt in range(NTT):
                emit_proj(tt)


    _split_excess_waits(nc)
    return nc


def _split_excess_waits(nc, maxw=1):
    """Walrus codegen rejects instructions carrying >1 sem wait; move excess
    waits onto no-ops inserted immediately before, on the same engine."""
    import concourse.mybir as mybir
    n = 0
    for f in nc.m.functions:
        for bb in f.blocks:
            new = []
            for inst in bb.instructions:
                si = getattr(inst, "sync_info", None)
                if si is not None and si.on_wait and len(si.on_wait) > maxw:
                    waits = list(si.on_wait)
                    excess, keep = waits[:-maxw], waits[-maxw:]
                    for i in range(0, len(excess), maxw):
                        new.append(mybir.InstNoOp(
                            name=f"{inst.name}_wsp{n}_{i}", engine=inst.engine,
                            bass_nofuse=True,
                            sync_info=mybir.SyncInfo(on_wait=excess[i:i + maxw],
                                                     on_update=[])))
                    si.on_wait = keep
                    n += 1
                new.append(inst)
            bb.instructions[:] = new
    return n


def _get_runner():
    """Build the Bass program once and wrap it in a shard_map-jitted callable
    over the 8 cores (mirrors concourse.bass2jax.run_bass_via_pjrt)."""
    if "runner" in _CACHE:
        return _CACHE["runner"]
    import jax
    import numpy as _np
    from jax.sharding import Mesh, PartitionSpec
    from jax.experimental.shard_map import shard_map
    import concourse.mybir as mybir
    from concourse.bass2jax import _bass_exec_p, install_neuronx_cc_hook

    install_neuronx_cc_hook()
    from concourse.bass2jax import partition_id_tensor
    nc = _build_program()

    part_name = nc.partition_id_tensor.name if nc.partition_id_tensor else None
    in_names, out_names, out_avals = [], [], []
    for alloc in nc.m.functions[0].allocations:
        if not isinstance(alloc, mybir.MemoryLocationSet):
            continue
        name = alloc.memorylocations[0].name
        if alloc.kind == "ExternalInput":
            if name != part_name:
                in_names.append(name)
        elif alloc.kind == "ExternalOutput":
            out_names.append(name)
            out_avals.append(jax.core.ShapedArray(
                tuple(alloc.tensor_shape), mybir.dt.np(alloc.dtype)))
    n_params = len(in_names)
    all_names = in_names + out_names
    if part_name is not None:
        all_names = all_names + [part_name]

    def _body(*args):
        operands = list(args)
        if part_name is not None:
            operands.append(partition_id_tensor())
        outs = _bass_exec_p.bind(
            *operands, out_avals=tuple(out_avals), in_names=tuple(all_names),
            out_names=tuple(out_names), lowering_input_output_aliases=(),
            sim_require_finite=True, sim_require_nnan=True, nc=nc)
        return tuple(outs)

    devices = jax.devices()[:N_CORES]
    mesh = Mesh(_np.asarray(devices), ("core",))
    n_outs = len(out_names)
    sharded = jax.jit(
        shard_map(_body, mesh=mesh,
                  in_specs=(PartitionSpec("core"),) * (n_params + n_outs),
                  out_specs=(PartitionSpec("core"),) * n_outs,
                  check_rep=False),
        donate_argnums=tuple(range(n_params, n_params + n_outs)),
        keep_unused=True)

    runner = (sharded, in_names, out_names, out_avals)
    _CACHE["runner"] = runner
    return runner


def _prepare_core_inputs(x, w_qkv, w_proj):
    bf = ml_dtypes.bfloat16
    cosT, sinT = _CACHE.setdefault("rope", _rope_tables())
    # q_rope = q*cos + R(q * sinP) with sinP = half-swapped sin:
    #   (R(q*sinP))[d] = sign_d * q[s(d)] * sinP[s(d)] = rot_half(q)[d] * sin[d]
    sinP = np.concatenate([sinT[D // 2:D], sinT[:D // 2]], axis=0)
    sinP = np.concatenate([sinP, sinP], axis=0)[:P]
    cosT, sinT = cosT.astype(bf), sinP.astype(bf)
    # lhsT for the on-device rotate-half matmul: out = rmat.T @ q = R_pair @ q
    R = np.zeros((D, D), np.float32)
    for d in range(D // 2):
        R[d, d + D // 2] = -1.0
        R[d + D // 2, d] = 1.0
    R_pair = np.zeros((P, P), np.float32)
    R_pair[:D, :D] = R
    R_pair[D:, D:] = R
    rmat = np.ascontiguousarray(R_pair.T).astype(bf)
    ident = np.eye(P, dtype=np.float32).astype(bf)
    # tri[j, i'] = 1 if i' >= j  (keep keys at or before the query)
    tri = np.triu(np.ones((P, P), np.float32)).astype(bf)
    xTs = [np.ascontiguousarray(x[b].T).astype(bf) for b in range(B)]
    per_core = []
    for core in range(N_CORES):
        b, g = divmod(core, 4)
        rows = slice(GC * g, GC * (g + 1))
        wq = w_qkv[0 * C:1 * C][rows]
        wk = w_qkv[1 * C:2 * C][rows]
        wv = w_qkv[2 * C:3 * C][rows]
        wT = np.ascontiguousarray(
            np.concatenate([wq, wk, wv], axis=0).T).astype(bf)      # [C, 768]
        wpT = np.ascontiguousarray(w_proj[:, rows].T).astype(bf)    # [256, C]
        per_core.append({
            "xT": xTs[b], "wT": wT, "wpT": wpT, "rmat": rmat,
            "ident": ident, "cosT": cosT, "sinT": sinT, "tri": tri})
    return per_core


def _run_cores(per_core):
    from concourse import bass_utils
    if "nc" not in _CACHE:
        from concourse.bass2jax import install_neuronx_cc_hook
        install_neuronx_cc_hook()
        _CACHE["nc"] = _build_program()
    res = bass_utils.run_bass_kernel_spmd(
        _CACHE["nc"], per_core, core_ids=list(range(N_CORES)))
    return res.results


def kernel(x, w_qkv, w_proj):
    x = np.asarray(x, dtype=np.float32)
    w_qkv = np.asarray(w_qkv, dtype=np.float32)
    w_proj = np.asarray(w_proj, dtype=np.float32)
    per_core = _prepare_core_inputs(x, w_qkv, w_proj)
    results = _run_cores(per_core)
    out = np.zeros((B, T, C), dtype=np.float32)
    for core in range(N_CORES):
        b = core // 4
        out[b] += np.asarray(results[core]["out"], dtype=np.float32)
    return out
